# revision 6
# baseline (speedup 1.0000x reference)
"""Trainium2 Bass kernel for ErnieImageAttention (non-causal MHA with per-head
RMSNorm on q/k + rotary embedding), tensor-parallel over heads on 8 NeuronCores.

Sharding: 16 heads / 8 cores = 2 heads per core. Each core computes its heads'
q/k/v projections, attention, and a partial output projection (row-parallel
Wo); the host sums the 8 partials and adds the bias.

Per-core dataflow (S=4096, D=2048, Hd=128, 2 local heads):
  phase 1 (per 128-row s-tile):
    q/k/v = hiddenT-chunk matmuls (f32r, N=256 = both heads) accumulated in PSUM
    RMSNorm stats from q/k (Square on ACT, free-dim reduce, sqrt, reciprocal)
    RoPE via host-precomputed coefficient tables (g and 1/sqrt(Hd) folded in)
    PE-transpose q/k 128x128 tiles into [d, s] layout (bf16), v kept [s, d] bf16
  phase 2 (per 512-col q-block, per head):
    scoresT[k,q] = kT.T @ qT (bf16 matmuls, k-tile stationary)
    expT = Exp(scoresT * r_k) on ACT (per-partition scale = deferred k-norm)
    denominator: DVE-accumulate expT tiles, then ones-vector matmul reduces
    partitions; reciprocal; rank-1 ones matmul broadcasts it back to 128 parts
    attn_T[d,q] = sum_k V[k,d]^T expT[k,q] accumulated in PSUM (bf16 matmuls)
    normalize: attn_T * bcast(1/denom)
  phase 3 (inline per q-block): fin[s, :2048] = sum_h attnT_h.T @ WoT_h (f32r)

The kernel is numerically softmax-stable without max-subtraction: logits are
~N(0,1) by construction (RMSNorm'd q/k, 1/sqrt(Hd) folded into q).
"""

import numpy as np

import concourse.bass as bass
import concourse.tile as tile
from concourse import bacc, mybir
from concourse import bass_utils
from concourse.masks import make_identity

F32 = mybir.dt.float32
F32R = mybir.dt.float32r
BF16 = mybir.dt.bfloat16
AX = mybir.AxisListType
AF = mybir.ActivationFunctionType

S = 4096
D = 2048
HD = 128
HEADS = 16
NCORES = 8
HLOC = HEADS // NCORES  # 2 heads per core
DLOC = HLOC * HD  # 256 local head dims
CH = D // 128  # 16 contraction chunks for projections
EPS = 1e-5
SCL = 1.0 / np.sqrt(HD)

QCOLS = 512  # q columns per attention block


def r32(ap):
    return ap.bitcast(F32R)


def build(nc, tc, io, s_len):
    st_n = s_len // 128  # s tiles
    qb_n = s_len // QCOLS  # q blocks
    qb_st = QCOLS // 128  # s tiles per q block
    kt_n = st_n  # k tiles

    ht, wq, wk, wv, wo, cgq, sgq, cgk, sgk, out = (
        io["ht"], io["wq"], io["wk"], io["wv"], io["wo"],
        io["cgq"], io["sgq"], io["cgk"], io["sgk"], io["out"],
    )

    import contextlib

    with contextlib.ExitStack() as ctx:
        ctx.enter_context(nc.allow_low_precision(
            reason="f32r operands for reduced-precision matmul; values are "
                   "O(1) and the rel-err budget is 2e-2"))
        consts = ctx.enter_context(tc.tile_pool(name="consts", bufs=1))
        persist = ctx.enter_context(tc.tile_pool(name="persist", bufs=1))
        ht_pool = ctx.enter_context(tc.tile_pool(name="ht", bufs=2))
        cs_pool = ctx.enter_context(tc.tile_pool(name="cs", bufs=2))
        work = ctx.enter_context(tc.tile_pool(name="work", bufs=2))
        et_pool = ctx.enter_context(tc.tile_pool(name="et", bufs=12))
        at_pool = ctx.enter_context(tc.tile_pool(name="at", bufs=4))
        araw_pool = ctx.enter_context(tc.tile_pool(name="araw", bufs=2))
        acc_pool = ctx.enter_context(tc.tile_pool(name="acc", bufs=2))
        rc_pool = ctx.enter_context(tc.tile_pool(name="rc", bufs=2))
        fin_pool = ctx.enter_context(tc.tile_pool(name="fin", bufs=4))

        # constants
        ident = consts.tile([128, 128], F32)
        make_identity(nc, ident[:])
        ones_f32 = consts.tile([128, 1], F32)
        nc.vector.memset(ones_f32[:], 1.0)
        ones_col = consts.tile([128, 1], F32R)
        nc.vector.tensor_copy(ones_col[:], ones_f32[:])
        ones_row_f32 = consts.tile([1, 128], F32)
        nc.vector.memset(ones_row_f32[:], 1.0)
        ones_row = consts.tile([1, 128], F32R)
        nc.vector.tensor_copy(ones_row[:], ones_row_f32[:])
        eps_t = consts.tile([128, 1], F32)
        nc.vector.memset(eps_t[:], EPS)

        wq_sb = consts.tile([128, CH, DLOC], F32R)
        nc.sync.dma_start(out=wq_sb[:], in_=wq)
        wk_sb = consts.tile([128, CH, DLOC], F32R)
        nc.sync.dma_start(out=wk_sb[:], in_=wk)
        wv_sb = consts.tile([128, CH, DLOC], F32R)
        nc.sync.dma_start(out=wv_sb[:], in_=wv)
        wo_sb = consts.tile([128, HLOC, D], F32R)
        nc.sync.dma_start(out=wo_sb[:], in_=wo)

        # persistent per-head transposed q/k, v, and deferred k-norm scales
        qT_sb = persist.tile([128, HLOC, st_n, 128], BF16)
        kT_sb = persist.tile([128, HLOC, st_n, 128], BF16)
        v_sb = persist.tile([128, st_n, DLOC], BF16)
        rk_sb = persist.tile([128, st_n, HLOC], F32)

        # ---------------- phase 1: projections + norm + rope + transpose ----
        with tc.tile_pool(name="ps1", bufs=2, space="PSUM") as ps1:
            for st in range(st_n):
                ht_t = ht_pool.tile([128, CH, 128], F32R, tag="ht")
                nc.sync.dma_start(out=ht_t[:], in_=ht[st])
                cgq_t = cs_pool.tile([128, HD], F32, tag="cgq")
                nc.sync.dma_start(out=cgq_t[:], in_=cgq[st * 128:(st + 1) * 128, :])
                sgq_t = cs_pool.tile([128, HD], F32, tag="sgq")
                nc.sync.dma_start(out=sgq_t[:], in_=sgq[st * 128:(st + 1) * 128, :])
                cgk_t = cs_pool.tile([128, HD], F32, tag="cgk")
                nc.sync.dma_start(out=cgk_t[:], in_=cgk[st * 128:(st + 1) * 128, :])
                sgk_t = cs_pool.tile([128, HD], F32, tag="sgk")
                nc.sync.dma_start(out=sgk_t[:], in_=sgk[st * 128:(st + 1) * 128, :])

                pq = ps1.tile([128, DLOC], F32, tag="pq")
                pk = ps1.tile([128, DLOC], F32, tag="pk")
                pv = ps1.tile([128, DLOC], F32, tag="pv")
                for c in range(CH):
                    lhs = ht_t[:, c, :]
                    nc.tensor.matmul(pq[:], lhs, wq_sb[:, c, :],
                                     start=(c == 0), stop=(c == CH - 1))
                    nc.tensor.matmul(pk[:], lhs, wk_sb[:, c, :],
                                     start=(c == 0), stop=(c == CH - 1))
                    nc.tensor.matmul(pv[:], lhs, wv_sb[:, c, :],
                                     start=(c == 0), stop=(c == CH - 1))

                # v: PSUM -> SBUF bf16
                nc.scalar.copy(v_sb[:, st, :], pv[:])

                # rms stats for q and k (mean of squares over each head's 128)
                for name, psrc, rdst in (("q", pq, None), ("k", pk, "rk")):
                    sq = work.tile([128, HLOC, HD], F32, tag=f"sq{name}")
                    nc.scalar.activation(sq[:], psrc[:].rearrange(
                        "p (h d) -> p h d", h=HLOC), AF.Square)
                    var = work.tile([128, HLOC], F32, tag=f"var{name}")
                    nc.vector.reduce_sum(out=var[:], in_=sq[:], axis=AX.X)
                    sig = work.tile([128, HLOC], F32, tag=f"sig{name}")
                    nc.scalar.activation(sig[:], var[:], AF.Sqrt,
                                         bias=eps_t[:], scale=1.0 / HD)
                    if rdst is None:
                        rq = work.tile([128, HLOC], F32, tag="rq")
                        nc.vector.reciprocal(rq[:], sig[:])
                    else:
                        nc.vector.reciprocal(rk_sb[:, st, :], sig[:])

                # rope q (scaled by rq and 1/sqrt(HD) via host tables) + rope k
                qa = work.tile([128, DLOC], F32, tag="qa")
                ka = work.tile([128, DLOC], F32, tag="ka")
                for h in range(HLOC):
                    hs = slice(h * HD, (h + 1) * HD)
                    # q: apply per-partition rq first, then rope
                    qs = work.tile([128, HD], F32, tag="qs")
                    nc.vector.tensor_scalar_mul(qs[:], pq[:, hs], rq[:, h:h + 1])
                    m1 = work.tile([128, HD], F32, tag="m1")
                    nc.vector.tensor_mul(m1[:], qs[:], cgq_t[:])
                    m2 = work.tile([128, HD], F32, tag="m2")
                    nc.vector.tensor_mul(m2[:, 0:64], qs[:, 64:128], sgq_t[:, 0:64])
                    nc.vector.tensor_mul(m2[:, 64:128], qs[:, 0:64], sgq_t[:, 64:128])
                    nc.vector.tensor_add(qa[:, hs], m1[:], m2[:])
                    # k: rope only (r_k deferred into exp scale)
                    m1k = work.tile([128, HD], F32, tag="m1k")
                    nc.vector.tensor_mul(m1k[:], pk[:, hs], cgk_t[:])
                    m2k = work.tile([128, HD], F32, tag="m2k")
                    nc.vector.tensor_mul(m2k[:, 0:64], pk[:, h * HD + 64:h * HD + 128],
                                         sgk_t[:, 0:64])
                    nc.vector.tensor_mul(m2k[:, 64:128], pk[:, h * HD:h * HD + 64],
                                         sgk_t[:, 64:128])
                    nc.vector.tensor_add(ka[:, hs], m1k[:], m2k[:])

                # transpose q/k tiles into [d, s] bf16
                for h in range(HLOC):
                    hs = slice(h * HD, (h + 1) * HD)
                    ptq = ps1.tile([128, 128], F32, tag="ptp")
                    nc.tensor.transpose(ptq[:], qa[:, hs], ident[:])
                    nc.scalar.copy(qT_sb[:, h, st, :], ptq[:])
                    ptk = ps1.tile([128, 128], F32, tag="ptp")
                    nc.tensor.transpose(ptk[:], ka[:, hs], ident[:])
                    nc.scalar.copy(kT_sb[:, h, st, :], ptk[:])

        # ---------------- phase 2+3: attention + output projection ----------
        with (
            tc.tile_pool(name="ps2", bufs=2, space="PSUM") as ps2,
            tc.tile_pool(name="ps2s", bufs=1, space="PSUM") as ps2s,
        ):
            for qb in range(qb_n):
                ats = []
                for h in range(HLOC):
                    acc = acc_pool.tile([128, QCOLS], F32R, tag="acc")
                    po = ps2.tile([128, QCOLS], F32, tag="po")
                    q_rhs = qT_sb[:, h, qb * qb_st:(qb + 1) * qb_st, :]
                    for kt in range(kt_n):
                        sc = ps2.tile([128, QCOLS], F32, tag="sc")
                        nc.tensor.matmul(sc[:], kT_sb[:, h, kt, :], q_rhs,
                                         start=True, stop=True)
                        et = et_pool.tile([128, QCOLS], BF16, tag="et")
                        nc.scalar.activation(et[:], sc[:], AF.Exp,
                                             scale=rk_sb[:, kt, h:h + 1])
                        if kt == 0:
                            nc.vector.tensor_copy(acc[:], et[:])
                        else:
                            nc.vector.tensor_add(acc[:], acc[:], et[:])
                        nc.tensor.matmul(po[:], v_sb[:, kt, h * HD:(h + 1) * HD],
                                         et[:], start=(kt == 0),
                                         stop=(kt == kt_n - 1))
                    # softmax denominator: partition-reduce, recip, broadcast
                    pd = ps2s.tile([1, QCOLS], F32, tag="pd")
                    nc.tensor.matmul(pd[:], ones_col[:], acc[:],
                                     start=True, stop=True)
                    recip = rc_pool.tile([1, QCOLS], F32R, tag="recip")
                    nc.vector.reciprocal(recip[:], pd[:])
                    pb = ps2s.tile([128, QCOLS], F32, tag="pb")
                    nc.tensor.matmul(pb[:], ones_row[:], recip[:],
                                     start=True, stop=True)
                    araw = araw_pool.tile([128, QCOLS], F32, tag="araw")
                    nc.scalar.copy(araw[:], po[:])
                    at = at_pool.tile([128, QCOLS], F32R, tag="at")
                    nc.vector.tensor_mul(at[:], araw[:], pb[:])
                    ats.append(at)

                # output projection for this q block
                for sti in range(qb_st):
                    st = qb * qb_st + sti
                    ss = slice(sti * 128, (sti + 1) * 128)
                    for nchunk in range(D // 512):
                        ns = slice(nchunk * 512, (nchunk + 1) * 512)
                        pf = ps2.tile([128, 512], F32, tag="pf")
                        for h in range(HLOC):
                            nc.tensor.matmul(pf[:], ats[h][:, ss],
                                             wo_sb[:, h, ns],
                                             start=(h == 0), stop=(h == HLOC - 1))
                        fin = fin_pool.tile([128, 512], F32, tag="fin")
                        nc.any.tensor_copy(fin[:], pf[:])
                        nc.sync.dma_start(
                            out=out[st * 128:(st + 1) * 128, ns], in_=fin[:])


def build_program(s_len=S):
    nc = bacc.Bacc("TRN2", target_bir_lowering=False, debug=False,
                   enable_asserts=False)
    st_n = s_len // 128
    io = {
        "ht": nc.dram_tensor("ht", [st_n, 128, CH, 128], F32R,
                             kind="ExternalInput").ap(),
        "wq": nc.dram_tensor("wq", [128, CH, DLOC], F32R,
                             kind="ExternalInput").ap(),
        "wk": nc.dram_tensor("wk", [128, CH, DLOC], F32R,
                             kind="ExternalInput").ap(),
        "wv": nc.dram_tensor("wv", [128, CH, DLOC], F32R,
                             kind="ExternalInput").ap(),
        "wo": nc.dram_tensor("wo", [128, HLOC, D], F32R,
                             kind="ExternalInput").ap(),
        "cgq": nc.dram_tensor("cgq", [s_len, HD], F32,
                              kind="ExternalInput").ap(),
        "sgq": nc.dram_tensor("sgq", [s_len, HD], F32,
                              kind="ExternalInput").ap(),
        "cgk": nc.dram_tensor("cgk", [s_len, HD], F32,
                              kind="ExternalInput").ap(),
        "sgk": nc.dram_tensor("sgk", [s_len, HD], F32,
                              kind="ExternalInput").ap(),
        "out": nc.dram_tensor("out", [s_len, D], F32,
                              kind="ExternalOutput").ap(),
    }
    with tile.TileContext(nc) as tc:
        build(nc, tc, io, s_len)
    nc.compile()
    return nc


def prep_inputs(inputs, s_len=S):
    """Host-side preprocessing: transposed/tiled layouts + rope coefficient
    tables (gains and the 1/sqrt(Hd) logit scale folded in)."""
    hs = np.asarray(inputs["hidden_states"], np.float32).reshape(s_len, D)
    st_n = s_len // 128
    ht = np.ascontiguousarray(
        hs.reshape(st_n, 128, CH, 128).transpose(0, 3, 2, 1))

    fc = np.asarray(inputs["freqs_cis"], np.float32).reshape(s_len, HD)
    cos = np.cos(fc)
    sin = np.sin(fc)
    gq = np.asarray(inputs["gq"], np.float32)
    gk = np.asarray(inputs["gk"], np.float32)

    def coef(g, scale):
        cg = cos * g[None, :] * scale
        sg = np.empty_like(sin)
        sg[:, :64] = -sin[:, :64] * g[None, 64:] * scale
        sg[:, 64:] = sin[:, 64:] * g[None, :64] * scale
        return np.ascontiguousarray(cg), np.ascontiguousarray(sg)

    cgq, sgq = coef(gq, SCL)
    cgk, sgk = coef(gk, 1.0)

    Wq = np.asarray(inputs["Wq"], np.float32)
    Wk = np.asarray(inputs["Wk"], np.float32)
    Wv = np.asarray(inputs["Wv"], np.float32)
    Wo = np.asarray(inputs["Wo"], np.float32)

    in_maps = []
    for c in range(NCORES):
        cols = slice(DLOC * c, DLOC * (c + 1))
        wq_c = np.ascontiguousarray(
            Wq[cols, :].T.reshape(CH, 128, DLOC).transpose(1, 0, 2))
        wk_c = np.ascontiguousarray(
            Wk[cols, :].T.reshape(CH, 128, DLOC).transpose(1, 0, 2))
        wv_c = np.ascontiguousarray(
            Wv[cols, :].T.reshape(CH, 128, DLOC).transpose(1, 0, 2))
        wo_c = np.ascontiguousarray(
            Wo[:, cols].T.reshape(HLOC, 128, D).transpose(1, 0, 2))
        in_maps.append({
            "ht": ht, "wq": wq_c, "wk": wk_c, "wv": wv_c, "wo": wo_c,
            "cgq": cgq, "sgq": sgq, "cgk": cgk, "sgk": sgk,
        })
    return in_maps


_CACHE = {}


def run_full(inputs, trace=False, **kw):
    if "nc" not in _CACHE:
        _CACHE["nc"] = build_program(S)
    nc = _CACHE["nc"]
    in_maps = prep_inputs(inputs, S)
    res = bass_utils.run_bass_kernel_spmd(
        nc, in_maps, core_ids=list(range(NCORES)), trace=trace, **kw)
    total = res.results[0]["out"].astype(np.float64)
    for c in range(1, NCORES):
        total += res.results[c]["out"]
    total += np.asarray(inputs["bo"], np.float64)[None, :]
    out = total.astype(np.float32).reshape(1, S, D)
    return out, res


def kernel(**inputs):
    out, _ = run_full(inputs, trace=False)
    return out
